# revision 1
# baseline (speedup 1.0000x reference)
"""CycleFC (per-channel width-shift + 1x1 conv) Trainium2 kernel.

Full shapes: x [32, 256, 56, 56] f32, weight [256, 256], bias [256].
out[b,o,h,w] = sum_c weight[o,c] * shift(x)[b,c,h,w] + bias[o]
where shift moves channel c along width by off(c) = (c+3)%7-3, zero-padded.

Strategy
--------
- Data-parallel over batch: 8 cores x 4 batches.
- Channels are permuted host-side so equal-shift channels ("classes")
  are contiguous, ordered so band widths DESCEND within each 128-channel
  group: g0 = [off 3 | off 2 | off 1 | off 0], g1 = [off -3 | -2 | -1 |
  off 0 rest]. The weight matrix is permuted to match (contraction is
  order-invariant), so no output un-permutation is needed.
- Flat-shift loads: each class segment is DMAed as per-channel fully
  CONTIGUOUS chunks (~12.5 KB) with the width-shift applied to the
  flattened h*w index: dst[p, f] = x[c, f+off]. The only wrong values
  land in the |off| <= 3 edge columns of each row (wrap from the
  neighboring row) plus the unwritten flat head/tail. Because band
  widths descend with partition index, the per-class zero-band is an
  affine staircase in (partition, col): ONE full-partition 3-column
  affine_select per tile zeroes exactly the wrap garbage. Compared to
  per-row trimmed windows this cuts DMA descriptors ~19x, keeps every
  descriptor >= 512 B (full DMA bus efficiency), and needs one SWDGE
  trigger (994 ns of Pool time) per segment instead of per row-piece.
- Matmul: out[o, hw] = lhsT.T @ rhs with lhsT = permuted weight.T
  [C, O] and rhs = shifted x tile [C, hw-chunk]. N-chunks of 448 (8
  rows), accumulated over the 2 channel groups in PSUM. Bias is fused
  into the PSUM->SBUF eviction on the scalar engine.

Semaphore-wait budget
---------------------
The walrus codegen encodes at most ONE semaphore wait per instruction.
- x tiles are unique and the class-segment DMAs are their FIRST
  writers, so load DMAs carry no data waits at all;
- the fixup select reads the band columns, which every segment DMA of
  the tile wrote, so the select depends on all 4 DMAs (multi-wait is
  resolved by probe selects, see _emit_fixup) and one PE absorber
  probing the select's output covers the whole tile transitively;
- real matmuls then wait only on PSUM-buffer recycling (ACT sem);
- evictions all run on the scalar engine so the 8 output stores (on 8
  fresh HWDGE lanes) wait on ACT alone.
"""

import numpy as np

B, C, O, H, W = 32, 256, 256, 56, 56
KS = 7
PAD = KS // 2
N_CORES = 8
B_LOC = B // N_CORES
HW = H * W
ROWS_PER_CHUNK = 8
N_FREE = ROWS_PER_CHUNK * W  # 448
N_CHUNKS = H // ROWS_PER_CHUNK  # 7
USE_F32R = False

# class order per group: band widths descend -> zero-band is an affine
# staircase per group (see _fixup below)
_CLS0 = [3, 2, 1]    # offs +3, +2, +1 (37 channels each), then off 0 head
_CLS1 = [4, 5, 6]    # offs -3, -2, -1 (36 channels each), then off 0 tail


def _perm_and_segments(n_chan=C):
    """Channel permutation and per-128-group DMA segments.

    Returns (perm, segs): segs[g] = list of (off, p0, p1, c0, c1) —
    partitions [p0,p1) of group g hold channels c0:c1:KS shifted by off.
    """
    assert n_chan == 256
    res = lambda r: list(range(r, n_chan, KS))
    off_of = lambda r: (r + PAD) % KS - PAD
    perm = []
    segs = [[], []]
    p = 0
    for r in _CLS0:
        ch = res(r)
        segs[0].append((off_of(r), p, p + len(ch), r, ch[-1] + 1))
        perm += ch
        p += len(ch)
    ch0 = res(0)
    head = ch0[:128 - p]
    segs[0].append((0, p, 128, head[0], head[-1] + 1))
    perm += head
    p = 0
    for r in _CLS1:
        ch = res(r)
        segs[1].append((off_of(r), p, p + len(ch), r, ch[-1] + 1))
        perm += ch
        p += len(ch)
    tail = ch0[len(head):]
    segs[1].append((0, p, 128, tail[0], tail[-1] + 1))
    perm += tail
    assert len(perm) == n_chan
    return np.array(perm), segs


def build_nc(b_loc=B_LOC, n_chan=C, n_out=O, h=H, w=W, rows_per_chunk=ROWS_PER_CHUNK,
             use_f32r=USE_F32R, psum_bufs=6):
    import concourse.bass as bass
    import concourse.mybir as mybir
    from concourse.tile import TileContext

    f32 = mybir.dt.float32
    xdt = mybir.dt.float32r if use_f32r else f32
    hw = h * w
    n_free = rows_per_chunk * w
    n_chunks = h // rows_per_chunk
    assert h % rows_per_chunk == 0
    n_groups = n_chan // 128
    o_groups = n_out // 128
    _, segs = _perm_and_segments(n_chan)

    nc = bass.Bass()
    x_d = nc.declare_dram_parameter("x", [b_loc, n_chan, hw], f32, isOutput=False)
    w_d = nc.declare_dram_parameter("wt", [n_chan, n_out], f32, isOutput=False)
    b_d = nc.declare_dram_parameter("bias", [128, o_groups], f32, isOutput=False)
    out_d = nc.declare_dram_parameter("out", [b_loc, n_out, h, w], f32, isOutput=True)

    from concourse.tile import add_dep_helper

    funnel = []

    with TileContext(nc) as tc:
        with (
            tc.tile_pool(name="const", bufs=1) as cpool,
            tc.tile_pool(name="xp", bufs=1) as xpool,
            tc.tile_pool(name="op", bufs=1) as opool,
            tc.tile_pool(name="ps", bufs=psum_bufs, space="PSUM") as pspool,
            tc.tile_pool(name="jk", bufs=1, space="PSUM") as jkpool,
        ):
            # --- constants (SWDGE; lanes reused -> own-lane wait only) ---
            wtiles = []
            for g in range(n_groups):
                wt = cpool.tile([128, n_out], xdt, tag=f"w{g}")
                nc.gpsimd.dma_start(out=wt[:], in_=w_d[g * 128:(g + 1) * 128, :])
                wtiles.append(wt)
            btile = cpool.tile([128, o_groups], f32, tag="bias")
            nc.gpsimd.dma_start(out=btile[:], in_=b_d[:])

            # --- PE absorbers --------------------------------------------
            jk = jkpool.tile([32, 512], f32, tag="junk")
            jk_col = [0]

            def absorb(lhsT, rhs, pos):
                nfree = rhs.shape[-1]
                c = jk_col[0]
                jk_col[0] = c + 2
                assert jk_col[0] <= 512
                m = min(lhsT.shape[-1], 32)
                nc.tensor.matmul(jk[0:m, c:c + nfree], lhsT, rhs, start=True,
                                 stop=True, skip_group_check=True,
                                 tile_position=(pos, 0))

            absorb(wtiles[0][0:32, 0:32], wtiles[0][0:32, 32:34], 0)
            absorb(wtiles[0][0:32, 0:32], wtiles[1][0:32, 0:2], 0)

            ajunk = cpool.tile([128, 4], f32, tag="ajunk")
            nc.scalar.activation(ajunk[0:32, 0:1], btile[0:32, 0:1],
                                 mybir.ActivationFunctionType.Identity)

            def sel_keep(win):
                # identity copy via affine_select (iota=0 -> all keep);
                # serves only to observe one segment DMA on Pool
                ncols = win.shape[-1]
                return nc.gpsimd.affine_select(
                    win, win, [[0, h], [0, ncols]],
                    mybir.AluOpType.is_ge, 0.0,
                    base=0, channel_multiplier=0)

            def fixup(xt3, g):
                # Probe keep-copies on growing partition prefixes [0:p1_k)
                # at DISJOINT interior columns: probe k's only unobserved
                # writer is segment k's DMA (earlier segs were observed by
                # probe k-1 on Pool; no write overlap -> no same-engine RAW
                # wait), so each carries exactly one semaphore wait. Then
                # ONE staircase select zeroes every class's wrap band with
                # all its DMA deps already observed on Pool.
                ps = [s[2] for s in segs[g]]
                for k, p1 in enumerate(ps):
                    sel_keep(xt3[0:p1, :, 40 + k:41 + k])
                if g == 0:
                    # bands: +3 [0,37) {53,54,55}; +2 [37,74) {54,55};
                    #        +1 [74,111) {55}; 0 [111,128) none
                    # keep iff p - 37*wi - 37 >= 0   (wi: col 53+wi)
                    win = xt3[:, :, w - PAD:w]
                    return nc.gpsimd.affine_select(
                        win, win, [[0, h], [-37, PAD]],
                        mybir.AluOpType.is_ge, 0.0,
                        base=-37, channel_multiplier=1)
                else:
                    # bands: -3 [0,36) {0,1,2}; -2 [36,72) {0,1};
                    #        -1 [72,108) {0}; 0 [108,128) none
                    # keep iff p + 36*wi - 108 >= 0  (wi: col wi)
                    win = xt3[:, :, 0:PAD]
                    return nc.gpsimd.affine_select(
                        win, win, [[0, h], [36, PAD]],
                        mybir.AluOpType.is_ge, 0.0,
                        base=-108, channel_multiplier=1)

            # --- main loop ----------------------------------------------
            sw_dmas = []
            last_mm = last_act = last_ms = None
            for b in range(b_loc):
                xts_b = []
                # hoist BOTH groups' seg DMAs ahead of the fixup selects:
                # the selects stall on DMA completion, and anything queued
                # behind them on Pool's SEQ (the next tile's triggers)
                # would stall too, starving the DMA engines
                for g in range(n_groups):
                    xt = xpool.tile([128, hw], xdt, tag=f"x{b}_{g}")
                    for (off, p0, p1, c0, c1) in segs[g]:
                        a, bb = max(0, -off), max(0, off)
                        d = nc.gpsimd.dma_start(
                            out=xt[p0:p1, a:hw - bb],
                            in_=x_d[b, c0:c1:KS, a + off:hw + off - bb])
                        sw_dmas.append(d)
                    xts_b.append(xt)
                for g in range(n_groups):
                    xt = xts_b[g]
                    xt3 = xt[:].rearrange("p (h w) -> p h w", w=w)
                    last_ms = fixup(xt3, g)
                    # PE absorbers, one per seg DMA: 32-wide spans in
                    # partition order so each span-read introduces exactly
                    # one not-yet-observed DMA (col 10 is outside every
                    # select window, so the DMAs are its only writers)
                    for s0 in (0, 32, 64, 96):
                        absorb(wtiles[0][s0:s0 + 32, 0:32],
                               xt[s0:s0 + 32, 10:12], s0)
                    # plus one probing an element whose ONLY writer is the
                    # staircase select (the flat tail/head the DMAs never
                    # cover): covers all Pool selects via the cumulative sem
                    col = (hw - 2) if g == 0 else 0
                    el = xt[32:33, col:col + 2]
                    absorb(el, el, 32)

                for og in range(o_groups):
                    ot = opool.tile([128, hw], f32, tag=f"ot{b}_{og}")
                    for n in range(n_chunks):
                        nsl = slice(n * n_free, (n + 1) * n_free)
                        ps = pspool.tile([128, n_free], f32, tag="ps")
                        for g in range(n_groups):
                            lhsT = wtiles[g][:, og * 128:(og + 1) * 128]
                            rhs = xts_b[g][:, nsl]
                            last_mm = nc.tensor.matmul(
                                ps[:], lhsT, rhs, start=(g == 0),
                                stop=(g == n_groups - 1))
                        last_act = nc.scalar.activation(
                            ot[:, nsl], ps[:],
                            mybir.ActivationFunctionType.Identity,
                            bias=btile[:, og:og + 1])
                    st = nc.sync.dma_start(
                        out=out_d[b, og * 128:(og + 1) * 128].rearrange(
                            "c h w -> c (h w)"),
                        in_=ot[:])
                    funnel.append(st)

            funnel.extend(sw_dmas[-8:])
            funnel.append(last_mm)
            funnel.append(last_act)
            funnel.append(last_ms)
            for dep in funnel:
                nop = nc.sync.nop(nofuse=True, hint="drain_funnel")
                add_dep_helper(nop.ins, dep.ins, reason="drain funnel")
    return nc


_CACHED_NC = None


def _get_nc():
    global _CACHED_NC
    if _CACHED_NC is None:
        _CACHED_NC = build_nc(use_f32r=USE_F32R)
    return _CACHED_NC


def run(x, weight, bias, trace=False):
    from concourse.bass_utils import run_bass_kernel_spmd

    perm, _ = _perm_and_segments(C)
    wt = np.ascontiguousarray(weight[:, perm].T)          # [C_perm, O]
    b2 = np.ascontiguousarray(bias.reshape(O // 128, 128).T)  # [128, o_groups]
    x = np.ascontiguousarray(x, dtype=np.float32).reshape(B, C, HW)

    nc = _get_nc()
    in_maps = [
        {"x": x[i * B_LOC:(i + 1) * B_LOC], "wt": wt, "bias": b2}
        for i in range(N_CORES)
    ]
    res = run_bass_kernel_spmd(nc, in_maps, list(range(N_CORES)), trace=trace)
    out = np.concatenate([res.results[i]["out"] for i in range(N_CORES)], axis=0)
    return out, res


def kernel(x, weight, bias):
    out, _ = run(x, weight, bias, trace=False)
    return out

